# revision 5
# baseline (speedup 1.0000x reference)
"""Bass/Trainium2 kernel for nn_BonsaiLayer (soft decision-tree layer).

Strategy (data-parallel over 8 NeuronCores, batch axis):
  - X is split host-side into fp16 planes Xh + Xl (Xl scaled by 2^11), giving
    ~22-bit effective precision for the branch-indicator matmul.
  - Host pre-transposes the planes so the device does plain contiguous DMAs
    (one 4MB DMA per plane per 2048-sample group) instead of transpose-DMAs.
  - One fused PE pass per 512-sample batch tile computes [Xp | a | c] where
    a = TZa@X-ish and c = -(TZb/S)@Xh; the branch indicator is the exact fp32
    compare u+ = (a > c) on VectorE (no sigmoid, no fp32 add).
  - u+ is PE-transposed (fp16) to batch-major; u- = 1-u+ via ScalarE affine.
  - Path probabilities built level by level on VectorE with node stride 64
    (padded) so every op runs at 2x fp16 rate.
  - W/V predictors run in fp16 with batch on PSUM partitions (columns padded
    630->640); tanh evacuation on ScalarE, W-product on VectorE via
    scalar_tensor_tensor straight from PSUM, probability multiply split
    between VectorE and GpSimd, one fused strided reduce per batch tile.
All shapes/strategy hardcoded for X[65536,1024], Z[64,1024], W/V[630,64], T[31,64].
"""
import sys
sys.path.insert(0, '/opt/trn_rl_repo')
import numpy as np
import concourse.bass as bass
import concourse.mybir as mybir
import concourse.tile as tile
from concourse import bacc
from concourse.bass_utils import run_bass_kernel_spmd
from concourse.masks import make_identity

F32, F16 = mybir.dt.float32, mybir.dt.float16
AF = mybir.ActivationFunctionType
OP = mybir.AluOpType

D, P, C, TOT, INT = 1024, 64, 10, 63, 31
NCORES = 8
B = 65536
BC = B // NCORES          # 8192 batch per core
NBT = BC // 512           # 16 batch tiles of 512
GRP = 4                   # batch tiles per staging group
S = 2048.0                # 2^11
Q = 64                    # node stride (TOT padded 63->64)
O_L = [0, 1, 3, 7, 15, 31]

_ordl = [[0]]
for _ in range(5):
    _ordl.append([2 * n + 1 for n in _ordl[-1]] + [2 * n + 2 for n in _ordl[-1]])
ORDINT = _ordl[0] + _ordl[1] + _ordl[2] + _ordl[3] + _ordl[4]
PERM = ORDINT + _ordl[5]

_nc_cache = None
_last_in_maps = None


def _build_nc(reps=1, loop_reps=None):
    nc = bacc.Bacc(None, target_bir_lowering=False)
    # host-transposed planes: [k, p, b] with b contiguous
    xh_d = nc.dram_tensor("xh", [8, 128, BC], F16, kind="ExternalInput")
    xl_d = nc.dram_tensor("xl", [8, 128, BC], F16, kind="ExternalInput")
    l1_d = nc.dram_tensor("l1", [8, 128, 128], F16, kind="ExternalInput")
    l2_d = nc.dram_tensor("l2", [8, 128, 128], F16, kind="ExternalInput")
    wv_d = nc.dram_tensor("wv", [64, 2 * C * Q], F16, kind="ExternalInput")
    out_d = nc.dram_tensor("out", [BC, C], F32, kind="ExternalOutput")

    GB = GRP * 512            # 2048 batch per staging group

    with tile.TileContext(nc) as tc:
        with tc.tile_pool(name="cst", bufs=1) as cst, \
             tc.tile_pool(name="stage", bufs=2) as stage, \
             tc.tile_pool(name="work", bufs=4) as work, \
             tc.tile_pool(name="work3", bufs=5) as work3, \
             tc.tile_pool(name="mps", bufs=2, space="PSUM") as mps, \
             tc.tile_pool(name="tps", bufs=2, space="PSUM") as tps, \
             tc.tile_pool(name="wps", bufs=1, space="PSUM") as wps:

            l1_sb = cst.tile([128, 8 * 128], F16)
            l2_sb = cst.tile([128, 8 * 128], F16)
            for k in range(8):
                nc.gpsimd.dma_start(l1_sb[:, k * 128:(k + 1) * 128], l1_d[k])
                nc.gpsimd.dma_start(l2_sb[:, k * 128:(k + 1) * 128], l2_d[k])
            wv_sb = cst.tile([64, 2 * C * Q], F16)
            nc.gpsimd.dma_start(wv_sb[:], wv_d[:, :])
            ident = cst.tile([INT, INT], F16)
            make_identity(nc, ident[:])
            score_sb = cst.tile([128, NBT * 4 * C], F32)

            import contextlib
            loop_ctx = tc.For_i(0, loop_reps, 1, hint_engines=tuple(nc.engines)) \
                if loop_reps else contextlib.nullcontext()
            with loop_ctx:
             for rep in range(reps):
              for g in range(NBT // GRP):
                  r0 = g * GB
                  sh = stage.tile([128, 8 * GB], F16, tag="sh")
                  sl = stage.tile([128, 8 * GB], F16, tag="sl")
                  nc.sync.dma_start(
                      sh[:].rearrange("p (k b) -> p k b", k=8),
                      xh_d[:, :, r0:r0 + GB].rearrange("k p b -> p k b"))
                  nc.sync.dma_start(
                      sl[:].rearrange("p (k b) -> p k b", k=8),
                      xl_d[:, :, r0:r0 + GB].rearrange("k p b -> p k b"))

                  for bt in range(GRP):
                      t0 = g * GRP + bt
                      bs = bt * 512
                      psm = mps.tile([128, 512], F32)
                      for k in range(8):
                          nc.tensor.matmul(psm[:], l1_sb[:, k * 128:(k + 1) * 128],
                                           sh[:, k * GB + bs:k * GB + bs + 512],
                                           start=(k == 0), stop=False)
                      for k in range(8):
                          nc.tensor.matmul(psm[:], l2_sb[:, k * 128:(k + 1) * 128],
                                           sl[:, k * GB + bs:k * GB + bs + 512],
                                           start=False, stop=(k == 7))

                      # Xp (fp16) for the W/V predictors
                      xph2 = work3.tile([64, 512], F16)
                      nc.scalar.copy(xph2[:], psm[0:64, :])

                      # branch indicator: u+ = (a > c), exact fp32 compare
                      c_sb = work.tile([INT, 512], F16)
                      nc.scalar.copy(c_sb[:], psm[96:127, :])
                      up_sb = work.tile([INT, 512], F16)
                      nc.vector.tensor_tensor(up_sb[:], psm[64:95, :], c_sb[:],
                                              OP.is_gt)

                      # transpose u+ to batch-major [128, 4j*32] (stride 32
                      # keeps PSUM writes 4B-aligned; pad col memset to 0)
                      upt = tps.tile([128, 128], F16)
                      upt3 = upt[:].rearrange("p (j n) -> p j n", j=4)
                      for j in range(4):
                          nc.tensor.transpose(upt[:, j * 32:j * 32 + INT],
                                              up_sb[:, j * 128:(j + 1) * 128],
                                              ident[:])
                      upm = work.tile([128, 256], F16)
                      upm3 = upm[:].rearrange("p (s j n) -> p s j n", s=2, j=4)
                      nc.scalar.copy(upm3[:, 0, :, 0:INT], upt3[:, :, 0:INT])
                      nc.scalar.activation(upm3[:, 1, :, 0:INT],
                                           upt3[:, :, 0:INT], AF.Copy,
                                           scale=-1.0, bias=1.0)

                      # path probabilities, level by level (node stride Q=64)
                      prb = work.tile([128, 4 * Q], F16)
                      p3 = prb[:].rearrange("p (j n) -> p j n", j=4)
                      nc.vector.memset(p3[:, :, 0:1], 1.0)
                      nc.vector.memset(p3[:, :, 63:64], 0.0)
                      u4 = upm[:].rearrange("p (s j n) -> p j s n", s=2, j=4,
                                            n=32)
                      for l in range(1, 6):
                          h = 2 ** (l - 1)
                          out_ap = p3[:, :, O_L[l]:O_L[l] + 2 * h].rearrange(
                              "p j (s i) -> p j s i", s=2)
                          in0 = p3[:, :, O_L[l - 1]:O_L[l - 1] + h].unsqueeze(2) \
                              .broadcast_to((128, 4, 2, h))
                          in1 = u4[:, :, :, O_L[l - 1]:O_L[l - 1] + h]
                          nc.vector.tensor_tensor(out_ap, in0, in1, OP.mult)

                      # predictors: per 128-sample chunk j
                      h_t = work3.tile([128, 4 * C * Q], F16)
                      for j in range(4):
                          wvpW = wps.tile([128, C * Q], F32, tag="wpsW")
                          wvpV = wps.tile([128, C * Q], F32, tag="wpsV")
                          lhsT = xph2[:, j * 128:(j + 1) * 128]
                          NW = C * Q
                          nc.tensor.matmul(wvpV[:, 0:512], lhsT,
                                           wv_sb[:, NW:NW + 512])
                          nc.tensor.matmul(wvpV[:, 512:NW], lhsT,
                                           wv_sb[:, NW + 512:2 * NW])
                          tnh = work3.tile([128, C * Q], F16)
                          nc.scalar.activation(tnh[:], wvpV[:], AF.Tanh,
                                               scale=1.0 / S)
                          nc.tensor.matmul(wvpW[:, 0:512], lhsT, wv_sb[:, 0:512])
                          nc.tensor.matmul(wvpW[:, 512:NW], lhsT,
                                           wv_sb[:, 512:NW])
                          g_t = work3.tile([128, C * Q], F16)
                          nc.vector.scalar_tensor_tensor(
                              g_t[:], wvpW[:], 1.0, tnh[:], OP.mult, OP.mult)
                          h3 = h_t[:, j * C * Q:(j + 1) * C * Q].rearrange(
                              "p (c q) -> p c q", c=C)
                          pb = prb[:, j * Q:(j + 1) * Q].unsqueeze(1) \
                              .broadcast_to((128, C, Q))
                          eng = nc.gpsimd if j == 3 else nc.vector
                          eng.tensor_tensor(
                              h3, g_t[:].rearrange("p (c q) -> p c q", c=C),
                              pb, OP.mult)
                      nc.vector.tensor_reduce(
                          score_sb[:, t0 * 4 * C:(t0 + 1) * 4 * C],
                          h_t[:].rearrange("p (j c q) -> p j c q", j=4, c=C),
                          axis=mybir.AxisListType.X, op=OP.add)

            nc.sync.dma_start(out_d.rearrange("(t p) c -> p t c", p=128),
                              score_sb[:].rearrange("p (t c) -> p t c", c=C))
    nc.finalize()
    return nc


def _get_nc():
    global _nc_cache
    if _nc_cache is None:
        _nc_cache = _build_nc()
    return _nc_cache


def kernel(X, Z, W, V, T):
    X = np.ascontiguousarray(np.asarray(X, dtype=np.float32))
    Z = np.asarray(Z, dtype=np.float64)
    W = np.asarray(W, dtype=np.float64)
    V = np.asarray(V, dtype=np.float64)
    T = np.asarray(T, dtype=np.float64)

    Zs = Z / P
    TZ = T[ORDINT] @ Zs                                   # [31, D]
    TZa = (TZ * S).astype(np.float16)
    TZb = ((TZ * S - TZa.astype(np.float64)) * S).astype(np.float16)
    L1 = np.zeros((D, 128), np.float16)
    L2 = np.zeros((D, 128), np.float16)
    L1[:, 0:64] = (Zs * S).astype(np.float16).T
    L1[:, 64:95] = TZa.T
    L1[:, 96:127] = -(TZb.astype(np.float64) / S).astype(np.float16).T
    L2[:, 0:64] = Zs.astype(np.float16).T
    L2[:, 64:95] = (TZa.astype(np.float64) / S).astype(np.float16).T
    LS1 = np.ascontiguousarray(L1.reshape(8, 128, 128))
    LS2 = np.ascontiguousarray(L2.reshape(8, 128, 128))

    # W/V packed [P, C, Q] with node column padded 63->64 (zeros)
    W3 = W.reshape(TOT, C, P)
    V3 = V.reshape(TOT, C, P)
    Wt = np.zeros((P, C, Q), np.float16)
    Vt = np.zeros((P, C, Q), np.float16)
    Wt[:, :, 0:TOT] = W3[PERM].transpose(2, 1, 0).astype(np.float16)
    Vt[:, :, 0:TOT] = V3[PERM].transpose(2, 1, 0).astype(np.float16)
    WVt = np.concatenate([Wt.reshape(P, C * Q), Vt.reshape(P, C * Q)], axis=1)

    Xh = X.astype(np.float16)
    Xl = ((X - Xh.astype(np.float32)) * np.float32(S)).astype(np.float16)

    in_maps = []
    for c in range(NCORES):
        sl = slice(c * BC, (c + 1) * BC)
        # transposed plane layout [k, p, b]: element = X[b, 128k + p]
        xh_c = np.ascontiguousarray(Xh[sl].T.reshape(8, 128, BC))
        xl_c = np.ascontiguousarray(Xl[sl].T.reshape(8, 128, BC))
        in_maps.append({"xh": xh_c, "xl": xl_c, "l1": LS1, "l2": LS2,
                        "wv": WVt})

    global _last_in_maps
    _last_in_maps = in_maps
    nc = _get_nc()
    res = run_bass_kernel_spmd(nc, in_maps, core_ids=list(range(NCORES)))
    score = np.concatenate([r["out"] for r in res.results], axis=0)  # [B, C]
    return np.ascontiguousarray((score.T * np.float32(1.0 / S)).astype(np.float32))


# revision 16
# speedup vs baseline: 1.3589x; 1.3589x over previous
"""Bass/Trainium2 kernel for nn_BonsaiLayer (soft decision-tree layer).

Strategy (data-parallel over 8 NeuronCores, batch axis):
  - X is split host-side into fp16 planes Xh + Xl (Xl scaled by 2^11), giving
    ~22-bit effective precision for the branch-indicator matmul.
  - Host pre-transposes the planes so the device does plain contiguous DMAs
    (one 4MB DMA per plane per 2048-sample group) instead of transpose-DMAs.
  - One fused PE pass per 512-sample batch tile computes [Xp | a | c] where
    a = TZa@X-ish and c = -(TZb/S)@Xh; the branch indicator is the exact fp32
    compare u+ = (a > c) on VectorE (no sigmoid, no fp32 add).
  - u+ is PE-transposed (fp16) to batch-major; u- = 1-u+ via ScalarE affine.
  - Path probabilities built level by level on VectorE with node stride 64
    (padded) so every op runs at 2x fp16 rate.
  - W/V predictors run in fp16 with batch on PSUM partitions (columns padded
    630->640); tanh evacuation on ScalarE, W-product on VectorE via
    scalar_tensor_tensor straight from PSUM, probability multiply split
    between VectorE and GpSimd, one fused strided reduce per batch tile.
All shapes/strategy hardcoded for X[65536,1024], Z[64,1024], W/V[630,64], T[31,64].
"""
import sys
sys.path.insert(0, '/opt/trn_rl_repo')
import numpy as np
import concourse.bass as bass
import concourse.mybir as mybir
import concourse.tile as tile
from concourse import bacc
from concourse.bass_utils import run_bass_kernel_spmd
from concourse.masks import make_identity

F32, F16 = mybir.dt.float32, mybir.dt.float16
AF = mybir.ActivationFunctionType
OP = mybir.AluOpType

D, P, C, TOT, INT = 1024, 64, 10, 63, 31
NCORES = 8
B = 65536
BC = B // NCORES          # 8192 batch per core
NBT = BC // 512           # 16 batch tiles of 512
GRP = 4                   # batch tiles per staging group
S = 2048.0                # 2^11
Q = 64                    # node stride (TOT padded 63->64)
O_L = [0, 1, 3, 7, 15, 31]

_ordl = [[0]]
for _ in range(5):
    _ordl.append([2 * n + 1 for n in _ordl[-1]] + [2 * n + 2 for n in _ordl[-1]])
ORDINT = _ordl[0] + _ordl[1] + _ordl[2] + _ordl[3] + _ordl[4]
PERM = ORDINT + _ordl[5]

_nc_cache = None
_last_in_maps = None


def _build_nc(reps=1, loop_reps=None):
    nc = bacc.Bacc(None, target_bir_lowering=False)
    # host-transposed planes: [k, p, b] with b contiguous
    xh_d = nc.dram_tensor("xh", [8, 128, BC], F16, kind="ExternalInput")
    xl_d = nc.dram_tensor("xl", [8, 128, BC], F16, kind="ExternalInput")
    l1_d = nc.dram_tensor("l1", [8, 128, 128], F16, kind="ExternalInput")
    l2_d = nc.dram_tensor("l2", [8, 128, 128], F16, kind="ExternalInput")
    wv_d = nc.dram_tensor("wv", [64, 2 * C * Q], F16, kind="ExternalInput")
    out_d = nc.dram_tensor("out", [BC, C], F32, kind="ExternalOutput")

    GB = GRP * 512            # 2048 batch per staging group

    with tile.TileContext(nc) as tc:
        with tc.tile_pool(name="cst", bufs=1) as cst, \
             tc.tile_pool(name="stage", bufs=2) as stage, \
             tc.tile_pool(name="work", bufs=4) as work, \
             tc.tile_pool(name="work3", bufs=3) as work3, \
             tc.tile_pool(name="mps", bufs=2, space="PSUM") as mps, \
             tc.tile_pool(name="tps", bufs=2, space="PSUM") as tps, \
             tc.tile_pool(name="wps", bufs=1, space="PSUM") as wps:

            l1_sb = cst.tile([128, 8 * 128], F16)
            l2_sb = cst.tile([128, 8 * 128], F16)
            for k in range(8):
                nc.gpsimd.dma_start(l1_sb[:, k * 128:(k + 1) * 128], l1_d[k])
                nc.gpsimd.dma_start(l2_sb[:, k * 128:(k + 1) * 128], l2_d[k])
            wv_sb = cst.tile([64, 2 * C * Q], F16)
            nc.gpsimd.dma_start(wv_sb[:], wv_d[:, :])
            ident = cst.tile([INT, INT], F16)
            make_identity(nc, ident[:])
            score_sb = cst.tile([128, NBT * 4 * C], F32)
            # persistent path-probability tiles: root=1 / pad=0 slots are
            # written once here; the tree only ever writes slots 1..62.
            prb_tiles = []
            for i in range(4):
                pt = cst.tile([128, 4 * Q], F16, tag=f"prb{i}")
                pt3 = pt[:].rearrange("p (j n) -> p j n", j=4)
                nc.vector.memset(pt3[:, :, 0:1], 1.0)
                nc.vector.memset(pt3[:, :, 63:64], 0.0)
                prb_tiles.append(pt)

            import contextlib
            loop_ctx = tc.For_i(0, loop_reps, 1, hint_engines=tuple(nc.engines)) \
                if loop_reps else contextlib.nullcontext()
            with loop_ctx:
             for rep in range(reps):
              for g in range(NBT // GRP):
                  r0 = g * GB
                  sh = stage.tile([128, 8 * GB], F16, tag="sh")
                  sl = stage.tile([128, 8 * GB], F16, tag="sl")
                  nc.sync.dma_start(
                      sh[:].rearrange("p (k b) -> p k b", k=8),
                      xh_d[:, :, r0:r0 + GB].rearrange("k p b -> p k b"))
                  nc.sync.dma_start(
                      sl[:].rearrange("p (k b) -> p k b", k=8),
                      xl_d[:, :, r0:r0 + GB].rearrange("k p b -> p k b"))

                  for bt in range(GRP):
                      t0 = g * GRP + bt
                      bs = bt * 512
                      psm = mps.tile([128, 512], F32)
                      for k in range(8):
                          nc.tensor.matmul(psm[:], l1_sb[:, k * 128:(k + 1) * 128],
                                           sh[:, k * GB + bs:k * GB + bs + 512],
                                           start=(k == 0), stop=False)
                      for k in range(8):
                          nc.tensor.matmul(psm[:], l2_sb[:, k * 128:(k + 1) * 128],
                                           sl[:, k * GB + bs:k * GB + bs + 512],
                                           start=False, stop=(k == 7))

                      # Xp (fp16) for the W/V predictors
                      xph2 = work3.tile([64, 512], F16)
                      nc.scalar.copy(xph2[:], psm[0:64, :])

                      # branch indicator: u+ = (a > c), exact fp32 compare
                      # (TensorTensor may read only one input from PSUM, so c
                      # is evacuated first; fp16 c is exact enough: it is a
                      # 2^-11-scale correction to a)
                      c_sb = work.tile([INT, 512], F16)
                      nc.scalar.copy(c_sb[:], psm[96:127, :])
                      up_sb = work.tile([INT, 512], F16)
                      nc.vector.tensor_tensor(up_sb[:], psm[64:95, :], c_sb[:],
                                              OP.is_gt)

                      # transpose u+ to batch-major [128, 4j*32] (stride 32
                      # keeps PSUM writes 4B-aligned; pad col memset to 0)
                      upt = tps.tile([128, 128], F16)
                      upt3 = upt[:].rearrange("p (j n) -> p j n", j=4)
                      for j in range(4):
                          nc.tensor.transpose(upt[:, j * 32:j * 32 + INT],
                                              up_sb[:, j * 128:(j + 1) * 128],
                                              ident[:])
                      upm = work.tile([128, 256], F16)
                      upm3 = upm[:].rearrange("p (s j n) -> p s j n", s=2, j=4)
                      nc.scalar.copy(upm3[:, 0, :, 0:INT], upt3[:, :, 0:INT])
                      nc.gpsimd.tensor_scalar(upm3[:, 1, :, 0:INT],
                                              upm3[:, 0, :, 0:INT], -1.0, 1.0,
                                              OP.mult, OP.add)

                      # path probabilities, level by level (node stride Q=64)
                      prb = prb_tiles[t0 % 4]
                      p3 = prb[:].rearrange("p (j n) -> p j n", j=4)
                      u4 = upm[:].rearrange("p (s j n) -> p j s n", s=2, j=4,
                                            n=32)
                      for l in range(1, 6):
                          h = 2 ** (l - 1)
                          out_ap = p3[:, :, O_L[l]:O_L[l] + 2 * h].rearrange(
                              "p j (s i) -> p j s i", s=2)
                          in0 = p3[:, :, O_L[l - 1]:O_L[l - 1] + h].unsqueeze(2) \
                              .broadcast_to((128, 4, 2, h))
                          in1 = u4[:, :, :, O_L[l - 1]:O_L[l - 1] + h]
                          nc.vector.tensor_tensor(out_ap, in0, in1, OP.mult)

                      # predictors: per 128-sample chunk j.  W-evacuation route
                      # and prob-multiply engine chosen per j to balance
                      # ScalarE / VectorE / GpSimd load.
                      h_t = work3.tile([128, 4 * C * Q], F16)
                      for j in range(4):
                          wvpW = wps.tile([128, C * Q], F32, tag="wpsW")
                          wvpV = wps.tile([128, C * Q], F32, tag="wpsV")
                          lhsT = xph2[:, j * 128:(j + 1) * 128]
                          NW = C * Q
                          nc.tensor.matmul(wvpV[:, 0:512], lhsT,
                                           wv_sb[:, NW:NW + 512])
                          nc.tensor.matmul(wvpV[:, 512:NW], lhsT,
                                           wv_sb[:, NW + 512:2 * NW])
                          tnh = work3.tile([128, C * Q], F16)
                          nc.scalar.activation(tnh[:], wvpV[:], AF.Tanh,
                                               scale=1.0 / S)
                          nc.tensor.matmul(wvpW[:, 0:512], lhsT, wv_sb[:, 0:512])
                          nc.tensor.matmul(wvpW[:, 512:NW], lhsT,
                                           wv_sb[:, 512:NW])
                          g_t = work3.tile([128, C * Q], F16)
                          if j < 2:
                              wx_sb = work3.tile([128, C * Q], F16)
                              nc.scalar.copy(wx_sb[:], wvpW[:])
                              nc.vector.tensor_tensor(g_t[:], wx_sb[:], tnh[:],
                                                      OP.mult)
                          else:
                              nc.vector.scalar_tensor_tensor(
                                  g_t[:], wvpW[:], 1.0, tnh[:], OP.mult,
                                  OP.mult)
                          h3 = h_t[:, j * C * Q:(j + 1) * C * Q].rearrange(
                              "p (c q) -> p c q", c=C)
                          pb = prb[:, j * Q:(j + 1) * Q].unsqueeze(1) \
                              .broadcast_to((128, C, Q))
                          eng = nc.vector if j == 0 else nc.gpsimd
                          eng.tensor_tensor(
                              h3, g_t[:].rearrange("p (c q) -> p c q", c=C),
                              pb, OP.mult)
                      # node-sum: fold 64->32->16->8 (fp16 2x), then reduce 8
                      h4 = h_t[:].rearrange("p (j c q) -> p j c q", j=4, c=C)
                      f1 = work3.tile([128, 4 * C * 32], F16)
                      f14 = f1[:].rearrange("p (j c q) -> p j c q", j=4, c=C)
                      nc.vector.tensor_tensor(f14, h4[:, :, :, 0:32],
                                              h4[:, :, :, 32:64], OP.add)
                      f2 = work3.tile([128, 4 * C * 16], F16)
                      f24 = f2[:].rearrange("p (j c q) -> p j c q", j=4, c=C)
                      nc.gpsimd.tensor_tensor(f24, f14[:, :, :, 0:16],
                                              f14[:, :, :, 16:32], OP.add)
                      f3 = work3.tile([128, 4 * C * 8], F16)
                      f34 = f3[:].rearrange("p (j c q) -> p j c q", j=4, c=C)
                      nc.vector.tensor_tensor(f34, f24[:, :, :, 0:8],
                                              f24[:, :, :, 8:16], OP.add)
                      nc.vector.tensor_reduce(
                          score_sb[:, t0 * 4 * C:(t0 + 1) * 4 * C],
                          f34, axis=mybir.AxisListType.X, op=OP.add)

            nc.sync.dma_start(out_d.rearrange("(t p) c -> p t c", p=128),
                              score_sb[:].rearrange("p (t c) -> p t c", c=C))
    nc.finalize()
    return nc


def _get_nc():
    global _nc_cache
    if _nc_cache is None:
        _nc_cache = _build_nc()
    return _nc_cache


def kernel(X, Z, W, V, T):
    X = np.ascontiguousarray(np.asarray(X, dtype=np.float32))
    Z = np.asarray(Z, dtype=np.float64)
    W = np.asarray(W, dtype=np.float64)
    V = np.asarray(V, dtype=np.float64)
    T = np.asarray(T, dtype=np.float64)

    Zs = Z / P
    TZ = T[ORDINT] @ Zs                                   # [31, D]
    TZa = (TZ * S).astype(np.float16)
    TZb = ((TZ * S - TZa.astype(np.float64)) * S).astype(np.float16)
    L1 = np.zeros((D, 128), np.float16)
    L2 = np.zeros((D, 128), np.float16)
    L1[:, 0:64] = (Zs * S).astype(np.float16).T
    L1[:, 64:95] = TZa.T
    L1[:, 96:127] = -(TZb.astype(np.float64) / S).astype(np.float16).T
    L2[:, 0:64] = Zs.astype(np.float16).T
    L2[:, 64:95] = (TZa.astype(np.float64) / S).astype(np.float16).T
    LS1 = np.ascontiguousarray(L1.reshape(8, 128, 128))
    LS2 = np.ascontiguousarray(L2.reshape(8, 128, 128))

    # W/V packed [P, C, Q] with node column padded 63->64 (zeros)
    W3 = W.reshape(TOT, C, P)
    V3 = V.reshape(TOT, C, P)
    Wt = np.zeros((P, C, Q), np.float16)
    Vt = np.zeros((P, C, Q), np.float16)
    Wt[:, :, 0:TOT] = W3[PERM].transpose(2, 1, 0).astype(np.float16)
    Vt[:, :, 0:TOT] = V3[PERM].transpose(2, 1, 0).astype(np.float16)
    WVt = np.concatenate([Wt.reshape(P, C * Q), Vt.reshape(P, C * Q)], axis=1)

    Xh = X.astype(np.float16)
    Xl = ((X - Xh.astype(np.float32)) * np.float32(S)).astype(np.float16)

    in_maps = []
    for c in range(NCORES):
        sl = slice(c * BC, (c + 1) * BC)
        # transposed plane layout [k, p, b]: element = X[b, 128k + p]
        xh_c = np.ascontiguousarray(Xh[sl].T.reshape(8, 128, BC))
        xl_c = np.ascontiguousarray(Xl[sl].T.reshape(8, 128, BC))
        in_maps.append({"xh": xh_c, "xl": xl_c, "l1": LS1, "l2": LS2,
                        "wv": WVt})

    global _last_in_maps
    _last_in_maps = in_maps
    nc = _get_nc()
    res = run_bass_kernel_spmd(nc, in_maps, core_ids=list(range(NCORES)))
    score = np.concatenate([r["out"] for r in res.results], axis=0)  # [B, C]
    return np.ascontiguousarray((score.T * np.float32(1.0 / S)).astype(np.float32))
